# revision 30
# baseline (speedup 1.0000x reference)
"""Trainium2 Bass kernel for nn_DenoiserPairFeatures.

Math: the [n,n,219] feature tensor is a concat of one-hots (seq-sep 127,
dist-bins 30+30) plus zero blocks, so feats @ W.T + b collapses to 3 table
gathers + bias.  Gathers are realized on the TensorEngine as sign-step
matmuls: one-hot(idx) @ T == B0 + sum_k sign(idx - k + .5) * half-delta_k
with host-precomputed compensated cumulative bf16 tables (error does not
accumulate along the chain).  LayerNorm is fused: per-tile stats via
bn_stats/bn_aggr, applied as out = y*scale - (-bias) in one pass, with the
pair mask folded into the scale.  Rows with mask[i]==0 produce all-zero
output and are written via zero-tile DMA without compute.

Sharding: rows of the pair grid are distributed round-robin (actives first)
over 8 cores; each core runs the same SPMD program on its own row set.
"""

import os
import sys

sys.path.insert(0, "/opt/trn_rl_repo")

import numpy as np
import ml_dtypes

N = 1024
SEQ = 127          # seq-sep one-hot classes
NB = 30            # dist bins
C_OUT = 256
N_CORES = 8
JT = 8             # j-tiles per row (1024 / 128)
LN_EPS = 1e-5

BF16 = ml_dtypes.bfloat16

_PROGRAM_CACHE = {}
LAST_PROFILE = None  # set when KERNEL_TRACE=1


def _bf16_f64(x):
    return np.asarray(x, np.float64).astype(BF16).astype(np.float64)


def _comp_chain(T):
    """Compensated half-delta chain for sign-step gather, split hi+lo bf16.

    T: [M+1, C] float64 exact targets.  Returns (Ghi, Glo [M, C] float64 of
    bf16-representable values, Q [C] = column sums of Ghi+Glo).  Realized
    partial sums P(k) = 2*sum_{m<=k} (Ghi+Glo)[m] track T[k]-T[0] with
    non-accumulating ~bf16^2-level error.
    """
    M = T.shape[0] - 1
    C = T.shape[1]
    P = np.zeros(C, np.float64)
    Ghi = np.empty((M, C), np.float64)
    Glo = np.empty((M, C), np.float64)
    for k in range(1, M + 1):
        g = (T[k] - T[0] - P) * 0.5
        ghi = _bf16_f64(g)
        glo = _bf16_f64(g - ghi)
        Ghi[k - 1] = ghi
        Glo[k - 1] = glo
        P += 2.0 * (ghi + glo)
    return Ghi, Glo, (Ghi + Glo).sum(axis=0)


def _dist_bins(coords):
    """Bin indices exactly as the reference computes them (same jnp ops on
    the default backend, so borderline fp32 decisions match bit-for-bit)."""
    import jax.numpy as jnp

    edges = jnp.linspace(0.1, 3.0, NB - 1)
    x = jnp.asarray(np.asarray(coords, np.float32))
    diff = x[:, None, :] - x[None, :, :]
    d = jnp.sqrt(jnp.sum(jnp.square(diff), axis=-1) + 1e-10)
    return np.asarray(jnp.searchsorted(edges, d), dtype=np.int32)


def _build_tables(W, b):
    """Returns ga_hi, ga_lo [128, 256] and gb2 [116, 256] (bf16).

    y = FA @ (ga_hi + ga_lo) + FB2 @ gb2, where FA rows are
    [const1, sign-steps(sep) x126, const1] and FB2 rows are the 58 bin
    sign-steps duplicated (hi block rows 0..57, lo block rows 58..115).
    The const1 rows carry a 4-way bf16 split of the scalar offset B0.
    """
    W = np.asarray(W, np.float64)
    b = np.asarray(b, np.float64)
    Tsep = W[:, 0:SEQ].T.copy()            # [127, 256]
    Tt = W[:, SEQ:SEQ + NB].T.copy()       # [30, 256]
    Tsc = W[:, SEQ + NB:SEQ + 2 * NB].T.copy()
    Gsep_h, Gsep_l, Qsep = _comp_chain(Tsep)   # [126, 256]
    Gt_h, Gt_l, Qt = _comp_chain(Tt)           # [29, 256]
    Gsc_h, Gsc_l, Qsc = _comp_chain(Tsc)       # [29, 256]
    B0 = b + Tsep[0] + Tt[0] + Tsc[0] + Qsep + Qt + Qsc
    p1 = _bf16_f64(B0)
    p2 = _bf16_f64(B0 - p1)
    p3 = _bf16_f64(B0 - p1 - p2)
    p4 = _bf16_f64(B0 - p1 - p2 - p3)
    ga_hi = np.concatenate([p1[None], Gsep_h, p3[None]], axis=0)   # [128, 256]
    ga_lo = np.concatenate([p2[None], Gsep_l, p4[None]], axis=0)   # [128, 256]
    gb2 = np.concatenate([Gt_h, Gsc_h, Gt_l, Gsc_l], axis=0)       # [116, 256]
    return ga_hi.astype(BF16), ga_lo.astype(BF16), gb2.astype(BF16)


def _build_program(R, n_zero_rows):
    """Build + compile the SPMD program for R active row-slots."""
    key = (R, n_zero_rows)
    if key in _PROGRAM_CACHE:
        return _PROGRAM_CACHE[key]

    from concourse import bacc, mybir, tile

    dt = mybir.dt
    nc = bacc.Bacc("TRN2", target_bir_lowering=False, debug=False,
                   num_devices=N_CORES)

    gah_d = nc.dram_tensor("ga_hi", [128, C_OUT], dt.bfloat16, kind="ExternalInput").ap()
    gal_d = nc.dram_tensor("ga_lo", [128, C_OUT], dt.bfloat16, kind="ExternalInput").ap()
    gb_d = nc.dram_tensor("gb2", [116, C_OUT], dt.bfloat16, kind="ExternalInput").ap()
    lta_d = nc.dram_tensor("lta", [4, 128 * 128], dt.bfloat16, kind="ExternalInput").ap()
    rhsa_d = nc.dram_tensor("rhsa", [4, 1024], dt.bfloat16, kind="ExternalInput").ap()
    ltb_d = nc.dram_tensor("ltb", [2, 116], dt.bfloat16, kind="ExternalInput").ap()
    tbsc_d = nc.dram_tensor("tbsc", [2, 128 * 1024], dt.bfloat16, kind="ExternalInput").ap()
    biasa_d = nc.dram_tensor("biasa", [128, 1], dt.float32, kind="ExternalInput").ap()
    biasb_d = nc.dram_tensor("biasb", [116, 1], dt.float32, kind="ExternalInput").ap()
    pmt_d = nc.dram_tensor("pmt", [128, 1024], dt.float32, kind="ExternalInput").ap()
    out_d = nc.dram_tensor("out", [128, 1024, C_OUT], dt.float32, kind="ExternalOutput").ap()

    with tile.TileContext(nc) as tc:
        with (
            tc.tile_pool(name="const", bufs=1) as cpool,
            tc.tile_pool(name="fa", bufs=3) as fapool,
            tc.tile_pool(name="fb", bufs=3) as fbpool,
            tc.tile_pool(name="pbc", bufs=2, space="PSUM") as pbc,
            tc.tile_pool(name="py", bufs=4, space="PSUM") as pyp,
            tc.tile_pool(name="stat", bufs=3) as spool,
            tc.tile_pool(name="fin", bufs=3) as finpool,
            tc.tile_pool(name="ot", bufs=3) as opool,
        ):
            GAH = cpool.tile([128, C_OUT], dt.bfloat16)
            nc.sync.dma_start(out=GAH[:], in_=gah_d[:])
            GAL = cpool.tile([128, C_OUT], dt.bfloat16)
            nc.sync.dma_start(out=GAL[:], in_=gal_d[:])
            GB = cpool.tile([116, C_OUT], dt.bfloat16)
            nc.sync.dma_start(out=GB[:], in_=gb_d[:])
            RHSA = cpool.tile([4, 1024], dt.bfloat16)
            nc.sync.dma_start(out=RHSA[:], in_=rhsa_d[:])
            LTB = cpool.tile([2, 116], dt.bfloat16)
            nc.sync.dma_start(out=LTB[:], in_=ltb_d[:])
            LTA = cpool.tile([4, 128 * 128], dt.bfloat16)
            nc.sync.dma_start(out=LTA[:], in_=lta_d[:])
            BIASA = cpool.tile([128, 1], dt.float32)
            nc.sync.dma_start(out=BIASA[:], in_=biasa_d[:])
            BIASB = cpool.tile([116, 1], dt.float32)
            nc.sync.dma_start(out=BIASB[:], in_=biasb_d[:])
            PMT = cpool.tile([128, 1024], dt.float32)
            nc.sync.dma_start(out=PMT[:], in_=pmt_d[:])
            ZT = cpool.tile([128, JT * C_OUT], dt.float32)
            nc.vector.memset(ZT[:], 0.0)
            EPS = cpool.tile([128, 1], dt.float32)
            nc.vector.memset(EPS[:], LN_EPS)

            Sign = mybir.ActivationFunctionType.Sign
            Sqrt = mybir.ActivationFunctionType.Sqrt
            Ident = mybir.ActivationFunctionType.Identity
            mult = mybir.AluOpType.mult
            add = mybir.AluOpType.add

            for r in range(R):
                # ---- stage per-row bin slices from DRAM ----
                TBS = fapool.tile([2, 1024], dt.bfloat16, tag="tbs")
                nc.sync.dma_start(out=TBS[:], in_=tbsc_d[:, r * 1024:(r + 1) * 1024])

                # ---- broadcast matmuls + sign steps -> F matrices ----
                FA = fapool.tile([128, 1024], dt.bfloat16, tag="fa")
                FB = fbpool.tile([116, 1024], dt.bfloat16, tag="fb")
                PA = pbc.tile([128, 2, 512], dt.float32, tag="pbc")
                PB = pbc.tile([128, 2, 512], dt.float32, tag="pbc")
                for h in range(2):
                    nc.tensor.matmul(
                        PA[:, h, :], LTA[:, r * 128:(r + 1) * 128],
                        RHSA[:, h * 512:(h + 1) * 512], start=True, stop=True)
                    nc.tensor.matmul(
                        PB[0:116, h, :], LTB[:],
                        TBS[:, h * 512:(h + 1) * 512],
                        start=True, stop=True)
                nc.scalar.activation(FA[:], PA[:], Sign, bias=BIASA[:, 0:1])
                nc.scalar.activation(FB[:], PB[0:116, :, :], Sign,
                                     bias=BIASB[:, 0:1])

                # ---- main matmuls (bank-paired Y) + stats + apply ----
                MV = spool.tile([128, JT, 2], dt.float32, tag="mv")
                SD = finpool.tile([128, JT], dt.float32, tag="sd")
                BD = finpool.tile([128, JT], dt.float32, tag="bd")
                OT = opool.tile([128, JT * C_OUT], dt.float32, tag="ot")
                ypairs = []
                for jp in range(JT // 2):
                    Y2 = pyp.tile([128, 2, C_OUT], dt.float32, tag="y")
                    ypairs.append(Y2)
                    for s in range(2):
                        jc = 2 * jp + s
                        nc.tensor.matmul(
                            Y2[:, s, :], FA[:, jc * 128:(jc + 1) * 128], GAH[:],
                            start=True, stop=False)
                        nc.tensor.matmul(
                            Y2[:, s, :], FA[:, jc * 128:(jc + 1) * 128], GAL[:],
                            start=False, stop=False)
                        nc.tensor.matmul(
                            Y2[:, s, :], FB[:, jc * 128:(jc + 1) * 128], GB[:],
                            start=False, stop=True)
                    ST = spool.tile([128, 2, 6], dt.float32, tag="st")
                    nc.vector.bn_stats(ST[:, 0, :], Y2[:, 0, :])
                    nc.vector.bn_stats(ST[:, 1, :], Y2[:, 1, :])
                    nc.vector.bn_aggr(MV[:, 2 * jp, :], ST[:, 0, :])
                    nc.vector.bn_aggr(MV[:, 2 * jp + 1, :], ST[:, 1, :])

                    if jp % 2 == 1:
                        g0 = 2 * (jp - 1)   # first jc of the 4-tile group
                        g1 = g0 + 4
                        # scale = pm / sqrt(var+eps); bias2 = -mean*scale
                        T0 = finpool.tile([128, 4], dt.float32, tag="t0")
                        nc.scalar.activation(
                            T0[:], MV[:, g0:g1, 1], Sqrt, bias=EPS[:, 0:1])
                        T1 = finpool.tile([128, 4], dt.float32, tag="t1")
                        nc.vector.reciprocal(T1[:], T0[:])
                        nc.vector.tensor_tensor(
                            SD[:, g0:g1], T1[:],
                            PMT[:, r * JT + g0: r * JT + g1], op=mult)
                        T2 = finpool.tile([128, 4], dt.float32, tag="t2")
                        nc.vector.tensor_tensor(
                            T2[:], MV[:, g0:g1, 0], SD[:, g0:g1], op=mult)
                        nc.vector.tensor_scalar(
                            BD[:, g0:g1], T2[:], -1.0, None, op0=mult)
                        for j2 in range(g0, g1):
                            ysrc = ypairs[j2 // 2][:, j2 % 2, :]
                            odst = OT[:, j2 * C_OUT:(j2 + 1) * C_OUT]
                            if j2 % 2 == 0:
                                nc.vector.tensor_scalar(
                                    odst, ysrc,
                                    SD[:, j2:j2 + 1], BD[:, j2:j2 + 1],
                                    op0=mult, op1=add)
                            else:
                                nc.scalar.activation(
                                    odst, ysrc, Ident,
                                    bias=BD[:, j2:j2 + 1], scale=SD[:, j2:j2 + 1])
                nc.sync.dma_start(
                    out=out_d[r].rearrange("(jc p) o -> p jc o", p=128),
                    in_=OT[:].rearrange("p (jc o) -> p jc o", o=C_OUT))

            # ---- zero rows: plain DMA of a zero tile ----
            for r in range(R, 128):
                nc.sync.dma_start(
                    out=out_d[r].rearrange("(jc p) o -> p jc o", p=128),
                    in_=ZT[:].rearrange("p (jc o) -> p jc o", o=C_OUT))

    nc.compile()
    _PROGRAM_CACHE[key] = nc
    return nc


def _host_data(mask, x_t, x_sc, W, b):
    """Everything data-dependent: bins, tables, row assignment, per-core inputs."""
    mask = np.asarray(mask)
    m = mask.astype(np.float64)
    ga_hi, ga_lo, gb2 = _build_tables(W, b)
    tb = _dist_bins(x_t)       # [n, n] int32 in [0, 29]
    sb = _dist_bins(x_sc)

    order = np.argsort(~mask.astype(bool), kind="stable")  # actives first
    n_active = int(mask.astype(bool).sum())
    R = min(128, max(1, (n_active + N_CORES - 1) // N_CORES))

    cores = []
    for c in range(N_CORES):
        rows = np.asarray(order[c::N_CORES])  # 128 global row ids, actives first
        i_r = rows.astype(np.int64)

        lta = np.zeros((4, 128 * 128), np.float64)
        a = (i_r + 63) // 256
        bb = (i_r + 63) % 256
        for p in range(1, 127):
            lta[0, p::128] = a
            lta[1, p::128] = bb
            lta[2, p::128] = 1.0
            lta[3, p::128] = 1.0

        tbsc = np.zeros((2, 128 * 1024), np.float64)
        tbsc[0] = tb[rows].reshape(-1)
        tbsc[1] = sb[rows].reshape(-1)

        pmt = np.zeros((128, 1024), np.float32)
        mrow = m[rows]                                  # [128]
        mcol = m.reshape(JT, 128).T                     # [128, JT] col-part masks
        for r in range(128):
            pmt[:, r * JT:(r + 1) * JT] = mcol * mrow[r]

        cores.append({
            "ga_hi": np.ascontiguousarray(ga_hi),
            "ga_lo": np.ascontiguousarray(ga_lo),
            "gb2": np.ascontiguousarray(gb2),
            "lta": lta.astype(BF16),
            "rhsa": _const_rhsa(),
            "ltb": _const_ltb(),
            "tbsc": tbsc.astype(BF16),
            "biasa": _const_biasa(),
            "biasb": _const_biasb(),
            "pmt": pmt,
        })
    row_lists = [np.asarray(order[c::N_CORES]) for c in range(N_CORES)]
    return cores, row_lists, R


def _const_rhsa():
    j = np.arange(1024)
    r = np.stack([np.full(1024, 256.0), np.ones(1024),
                  -256.0 * (j // 256), -(j % 256).astype(np.float64)])
    return r.astype(BF16)


def _const_ltb():
    ltb = np.zeros((2, 116), np.float64)
    ltb[0, 0:29] = 1.0
    ltb[1, 29:58] = 1.0
    ltb[0, 58:87] = 1.0
    ltb[1, 87:116] = 1.0
    return ltb.astype(BF16)


def _const_biasa():
    v = np.empty((128, 1), np.float32)
    v[0, 0] = 1.0
    v[127, 0] = 1.0
    for k in range(1, 127):
        v[k, 0] = 0.5 - k
    return v


def _const_biasb():
    v = np.empty((116, 1), np.float32)
    for k in range(29):
        v[k, 0] = -(k + 0.5)
        v[29 + k, 0] = -(k + 0.5)
    v[58:116] = v[0:58]
    return v


def kernel(mask, x_t, x_sc, W, b, gamma, beta):
    global LAST_PROFILE
    from concourse.bass_utils import run_bass_kernel_spmd

    mask = np.asarray(mask)
    cores, row_lists, R = _host_data(mask, x_t, x_sc, W, b)
    nc = _build_program(R, 128 - R)

    trace = bool(int(os.environ.get("KERNEL_TRACE", "0")))
    res = run_bass_kernel_spmd(nc, cores, list(range(N_CORES)), trace=trace)
    LAST_PROFILE = res

    out = np.empty((N, N, C_OUT), np.float32)
    for c in range(N_CORES):
        out[row_lists[c]] = res.results[c]["out"]

    gamma = np.asarray(gamma, np.float32)
    beta = np.asarray(beta, np.float32)
    if not (np.all(gamma == 1.0) and np.all(beta == 0.0)):
        pm = (mask.astype(np.float32)[:, None] * mask.astype(np.float32)[None, :])
        out = out * gamma[None, None, :] + pm[:, :, None] * beta[None, None, :]
    return out


# revision 34
# speedup vs baseline: 1.1796x; 1.1796x over previous
"""Trainium2 Bass kernel for nn_DenoiserPairFeatures.

Math: the [n,n,219] feature tensor is a concat of one-hots (seq-sep 127,
dist-bins 30+30) plus zero blocks, so feats @ W.T + b collapses to 3 table
gathers + bias.  Gathers are realized on the TensorEngine as sign-step
matmuls: one-hot(idx) @ T == B0 + sum_k sign(idx - k + .5) * half-delta_k
with host-precomputed compensated cumulative bf16 tables (error does not
accumulate along the chain).  LayerNorm is fused: per-tile stats via
bn_stats/bn_aggr, applied as out = y*scale - (-bias) in one pass, with the
pair mask folded into the scale.  Rows with mask[i]==0 produce all-zero
output and are written via zero-tile DMA without compute.

Sharding: rows of the pair grid are distributed round-robin (actives first)
over 8 cores; each core runs the same SPMD program on its own row set.
"""

import os
import sys

sys.path.insert(0, "/opt/trn_rl_repo")

import numpy as np
import ml_dtypes

N = 1024
SEQ = 127          # seq-sep one-hot classes
NB = 30            # dist bins
C_OUT = 256
N_CORES = 8
JT = 8             # j-tiles per row (1024 / 128)
LN_EPS = 1e-5

BF16 = ml_dtypes.bfloat16

_PROGRAM_CACHE = {}
LAST_PROFILE = None  # set when KERNEL_TRACE=1


def _bf16_f64(x):
    return np.asarray(x, np.float64).astype(BF16).astype(np.float64)


def _comp_chain(T):
    """Compensated half-delta chain for sign-step gather, split hi+lo bf16.

    T: [M+1, C] float64 exact targets.  Returns (Ghi, Glo [M, C] float64 of
    bf16-representable values, Q [C] = column sums of Ghi+Glo).  Realized
    partial sums P(k) = 2*sum_{m<=k} (Ghi+Glo)[m] track T[k]-T[0] with
    non-accumulating ~bf16^2-level error.
    """
    M = T.shape[0] - 1
    C = T.shape[1]
    P = np.zeros(C, np.float64)
    Ghi = np.empty((M, C), np.float64)
    Glo = np.empty((M, C), np.float64)
    for k in range(1, M + 1):
        g = (T[k] - T[0] - P) * 0.5
        ghi = _bf16_f64(g)
        glo = _bf16_f64(g - ghi)
        Ghi[k - 1] = ghi
        Glo[k - 1] = glo
        P += 2.0 * (ghi + glo)
    return Ghi, Glo, (Ghi + Glo).sum(axis=0)


def _dist_bins(coords):
    """Bin indices exactly as the reference computes them (same jnp ops on
    the default backend, so borderline fp32 decisions match bit-for-bit)."""
    import jax.numpy as jnp

    edges = jnp.linspace(0.1, 3.0, NB - 1)
    x = jnp.asarray(np.asarray(coords, np.float32))
    diff = x[:, None, :] - x[None, :, :]
    d = jnp.sqrt(jnp.sum(jnp.square(diff), axis=-1) + 1e-10)
    return np.asarray(jnp.searchsorted(edges, d), dtype=np.int32)


def _build_tables(W, b):
    """Returns ga_hi, ga_lo [128, 256] and gb2 [116, 256] (bf16).

    y = FA @ (ga_hi + ga_lo) + FB2 @ gb2, where FA rows are
    [const1, sign-steps(sep) x126, const1] and FB2 rows are the 58 bin
    sign-steps duplicated (hi block rows 0..57, lo block rows 58..115).
    The const1 rows carry a 4-way bf16 split of the scalar offset B0.
    """
    W = np.asarray(W, np.float64)
    b = np.asarray(b, np.float64)
    Tsep = W[:, 0:SEQ].T.copy()            # [127, 256]
    Tt = W[:, SEQ:SEQ + NB].T.copy()       # [30, 256]
    Tsc = W[:, SEQ + NB:SEQ + 2 * NB].T.copy()
    Gsep_h, Gsep_l, Qsep = _comp_chain(Tsep)   # [126, 256]
    Gt_h, Gt_l, Qt = _comp_chain(Tt)           # [29, 256]
    Gsc_h, Gsc_l, Qsc = _comp_chain(Tsc)       # [29, 256]
    B0 = b + Tsep[0] + Tt[0] + Tsc[0] + Qsep + Qt + Qsc
    p1 = _bf16_f64(B0)
    p2 = _bf16_f64(B0 - p1)
    p3 = _bf16_f64(B0 - p1 - p2)
    p4 = _bf16_f64(B0 - p1 - p2 - p3)
    ga_hi = np.concatenate([p1[None], Gsep_h, p3[None]], axis=0)   # [128, 256]
    ga_lo = np.concatenate([p2[None], Gsep_l, p4[None]], axis=0)   # [128, 256]
    gb2 = np.concatenate([Gt_h, Gsc_h, Gt_l, Gsc_l], axis=0)       # [116, 256]
    return ga_hi.astype(BF16), ga_lo.astype(BF16), gb2.astype(BF16)


def _build_program(R, n_zero_rows):
    """Build + compile the SPMD program for R active row-slots."""
    key = (R, n_zero_rows)
    if key in _PROGRAM_CACHE:
        return _PROGRAM_CACHE[key]

    from concourse import bacc, mybir, tile

    dt = mybir.dt
    nc = bacc.Bacc("TRN2", target_bir_lowering=False, debug=False,
                   num_devices=N_CORES)

    gah_d = nc.dram_tensor("ga_hi", [128, C_OUT], dt.bfloat16, kind="ExternalInput").ap()
    gal_d = nc.dram_tensor("ga_lo", [128, C_OUT], dt.bfloat16, kind="ExternalInput").ap()
    gb_d = nc.dram_tensor("gb2", [116, C_OUT], dt.bfloat16, kind="ExternalInput").ap()
    lta_d = nc.dram_tensor("lta", [4, 128 * 128], dt.bfloat16, kind="ExternalInput").ap()
    rhsa_d = nc.dram_tensor("rhsa", [4, 1024], dt.bfloat16, kind="ExternalInput").ap()
    ltb_d = nc.dram_tensor("ltb", [2, 116], dt.bfloat16, kind="ExternalInput").ap()
    tbsc_d = nc.dram_tensor("tbsc", [2, 128 * 1024], dt.bfloat16, kind="ExternalInput").ap()
    biasa_d = nc.dram_tensor("biasa", [128, 1], dt.float32, kind="ExternalInput").ap()
    biasb_d = nc.dram_tensor("biasb", [116, 1], dt.float32, kind="ExternalInput").ap()
    pmt_d = nc.dram_tensor("pmt", [128, 1024], dt.float32, kind="ExternalInput").ap()
    out_d = nc.dram_tensor("out", [128, 1024, C_OUT], dt.float32, kind="ExternalOutput").ap()

    with tile.TileContext(nc) as tc:
        with (
            tc.tile_pool(name="const", bufs=1) as cpool,
            tc.tile_pool(name="fa", bufs=4) as fapool,
            tc.tile_pool(name="fb", bufs=4) as fbpool,
            tc.tile_pool(name="pbc", bufs=2, space="PSUM") as pbc,
            tc.tile_pool(name="py", bufs=6, space="PSUM") as pyp,
            tc.tile_pool(name="stat", bufs=8) as spool,
            tc.tile_pool(name="fin", bufs=6) as finpool,
            tc.tile_pool(name="ot", bufs=4) as opool,
        ):
            GAH = cpool.tile([128, C_OUT], dt.bfloat16)
            nc.sync.dma_start(out=GAH[:], in_=gah_d[:])
            GAL = cpool.tile([128, C_OUT], dt.bfloat16)
            nc.sync.dma_start(out=GAL[:], in_=gal_d[:])
            GB = cpool.tile([116, C_OUT], dt.bfloat16)
            nc.sync.dma_start(out=GB[:], in_=gb_d[:])
            RHSA = cpool.tile([4, 1024], dt.bfloat16)
            nc.sync.dma_start(out=RHSA[:], in_=rhsa_d[:])
            LTB = cpool.tile([2, 116], dt.bfloat16)
            nc.sync.dma_start(out=LTB[:], in_=ltb_d[:])
            LTA = cpool.tile([4, 128 * 128], dt.bfloat16)
            nc.sync.dma_start(out=LTA[:], in_=lta_d[:])
            BIASA = cpool.tile([128, 1], dt.float32)
            nc.sync.dma_start(out=BIASA[:], in_=biasa_d[:])
            BIASB = cpool.tile([116, 1], dt.float32)
            nc.sync.dma_start(out=BIASB[:], in_=biasb_d[:])
            PMT = cpool.tile([128, 1024], dt.float32)
            nc.sync.dma_start(out=PMT[:], in_=pmt_d[:])
            ZT = cpool.tile([128, JT * C_OUT], dt.float32)
            nc.vector.memset(ZT[:], 0.0)
            EPS = cpool.tile([128, 1], dt.float32)
            nc.vector.memset(EPS[:], LN_EPS)

            Sign = mybir.ActivationFunctionType.Sign
            Sqrt = mybir.ActivationFunctionType.Sqrt
            Ident = mybir.ActivationFunctionType.Identity
            mult = mybir.AluOpType.mult
            add = mybir.AluOpType.add

            for r in range(R):
                # ---- stage per-row bin slices from DRAM ----
                TBS = fapool.tile([2, 1024], dt.bfloat16, tag="tbs")
                nc.sync.dma_start(out=TBS[:], in_=tbsc_d[:, r * 1024:(r + 1) * 1024])

                # ---- broadcast matmuls + sign steps -> F matrices ----
                FA = fapool.tile([128, 1024], dt.bfloat16, tag="fa")
                FB = fbpool.tile([116, 1024], dt.bfloat16, tag="fb")
                for h in range(2):
                    PA = pbc.tile([128, 512], dt.float32, tag="pbc")
                    nc.tensor.matmul(
                        PA[:], LTA[:, r * 128:(r + 1) * 128],
                        RHSA[:, h * 512:(h + 1) * 512], start=True, stop=True)
                    nc.scalar.activation(
                        FA[:, h * 512:(h + 1) * 512], PA[:], Sign, bias=BIASA[:, 0:1])
                    PB = pbc.tile([128, 512], dt.float32, tag="pbc")
                    nc.tensor.matmul(
                        PB[0:116, :], LTB[:],
                        TBS[:, h * 512:(h + 1) * 512],
                        start=True, stop=True)
                    nc.scalar.activation(
                        FB[:, h * 512:(h + 1) * 512], PB[0:116, :], Sign,
                        bias=BIASB[:, 0:1])

                # ---- main matmuls (bank-paired Y) + stats + apply ----
                MV = spool.tile([128, JT, 2], dt.float32, tag="mv")
                SD = finpool.tile([128, JT], dt.float32, tag="sd")
                BD = finpool.tile([128, JT], dt.float32, tag="bd")
                OT = opool.tile([128, JT * C_OUT], dt.float32, tag="ot")
                ypairs = []
                for jp in range(JT // 2):
                    Y2 = pyp.tile([128, 2, C_OUT], dt.float32, tag="y")
                    ypairs.append(Y2)
                    for s in range(2):
                        jc = 2 * jp + s
                        nc.tensor.matmul(
                            Y2[:, s, :], FA[:, jc * 128:(jc + 1) * 128], GAH[:],
                            start=True, stop=False)
                        nc.tensor.matmul(
                            Y2[:, s, :], FA[:, jc * 128:(jc + 1) * 128], GAL[:],
                            start=False, stop=False)
                        nc.tensor.matmul(
                            Y2[:, s, :], FB[:, jc * 128:(jc + 1) * 128], GB[:],
                            start=False, stop=True)
                    ST = spool.tile([128, 2, 6], dt.float32, tag="st")
                    nc.vector.bn_stats(ST[:, 0, :], Y2[:, 0, :])
                    nc.vector.bn_stats(ST[:, 1, :], Y2[:, 1, :])
                    nc.vector.bn_aggr(MV[:, 2 * jp, :], ST[:, 0, :])
                    nc.vector.bn_aggr(MV[:, 2 * jp + 1, :], ST[:, 1, :])

                    if jp % 2 == 1:
                        g0 = 2 * (jp - 1)   # first jc of the 4-tile group
                        g1 = g0 + 4
                        # scale = pm / sqrt(var+eps); bias2 = -mean*scale
                        T0 = finpool.tile([128, 4], dt.float32, tag="t0")
                        nc.scalar.activation(
                            T0[:], MV[:, g0:g1, 1], Sqrt, bias=EPS[:, 0:1])
                        T1 = finpool.tile([128, 4], dt.float32, tag="t1")
                        nc.vector.reciprocal(T1[:], T0[:])
                        nc.vector.tensor_tensor(
                            SD[:, g0:g1], T1[:],
                            PMT[:, r * JT + g0: r * JT + g1], op=mult)
                        T2 = finpool.tile([128, 4], dt.float32, tag="t2")
                        nc.vector.tensor_tensor(
                            T2[:], MV[:, g0:g1, 0], SD[:, g0:g1], op=mult)
                        nc.vector.tensor_scalar(
                            BD[:, g0:g1], T2[:], -1.0, None, op0=mult)
                        for j2 in range(g0, g1):
                            ysrc = ypairs[j2 // 2][:, j2 % 2, :]
                            odst = OT[:, j2 * C_OUT:(j2 + 1) * C_OUT]
                            if j2 % 4 == 0:
                                nc.vector.tensor_scalar(
                                    odst, ysrc,
                                    SD[:, j2:j2 + 1], BD[:, j2:j2 + 1],
                                    op0=mult, op1=add)
                            else:
                                nc.scalar.activation(
                                    odst, ysrc, Ident,
                                    bias=BD[:, j2:j2 + 1], scale=SD[:, j2:j2 + 1])
                    if jp % 2 == 1:
                        half = jp // 2
                        nc.sync.dma_start(
                            out=out_d[r, half * 512:(half + 1) * 512, :]
                                .rearrange("(jc p) o -> p jc o", p=128),
                            in_=OT[:, half * 4 * C_OUT:(half + 1) * 4 * C_OUT]
                                .rearrange("p (jc o) -> p jc o", o=C_OUT))

            # ---- zero rows: plain DMA of a zero tile ----
            for r in range(R, 128):
                nc.sync.dma_start(
                    out=out_d[r].rearrange("(jc p) o -> p jc o", p=128),
                    in_=ZT[:].rearrange("p (jc o) -> p jc o", o=C_OUT))

    nc.compile()
    _PROGRAM_CACHE[key] = nc
    return nc


def _host_data(mask, x_t, x_sc, W, b):
    """Everything data-dependent: bins, tables, row assignment, per-core inputs."""
    mask = np.asarray(mask)
    m = mask.astype(np.float64)
    ga_hi, ga_lo, gb2 = _build_tables(W, b)
    tb = _dist_bins(x_t)       # [n, n] int32 in [0, 29]
    sb = _dist_bins(x_sc)

    order = np.argsort(~mask.astype(bool), kind="stable")  # actives first
    n_active = int(mask.astype(bool).sum())
    R = min(128, max(1, (n_active + N_CORES - 1) // N_CORES))

    cores = []
    for c in range(N_CORES):
        rows = np.asarray(order[c::N_CORES])  # 128 global row ids, actives first
        i_r = rows.astype(np.int64)

        lta = np.zeros((4, 128 * 128), np.float64)
        a = (i_r + 63) // 256
        bb = (i_r + 63) % 256
        for p in range(1, 127):
            lta[0, p::128] = a
            lta[1, p::128] = bb
            lta[2, p::128] = 1.0
            lta[3, p::128] = 1.0

        tbsc = np.zeros((2, 128 * 1024), np.float64)
        tbsc[0] = tb[rows].reshape(-1)
        tbsc[1] = sb[rows].reshape(-1)

        pmt = np.zeros((128, 1024), np.float32)
        mrow = m[rows]                                  # [128]
        mcol = m.reshape(JT, 128).T                     # [128, JT] col-part masks
        for r in range(128):
            pmt[:, r * JT:(r + 1) * JT] = mcol * mrow[r]

        cores.append({
            "ga_hi": np.ascontiguousarray(ga_hi),
            "ga_lo": np.ascontiguousarray(ga_lo),
            "gb2": np.ascontiguousarray(gb2),
            "lta": lta.astype(BF16),
            "rhsa": _const_rhsa(),
            "ltb": _const_ltb(),
            "tbsc": tbsc.astype(BF16),
            "biasa": _const_biasa(),
            "biasb": _const_biasb(),
            "pmt": pmt,
        })
    row_lists = [np.asarray(order[c::N_CORES]) for c in range(N_CORES)]
    return cores, row_lists, R


def _const_rhsa():
    j = np.arange(1024)
    r = np.stack([np.full(1024, 256.0), np.ones(1024),
                  -256.0 * (j // 256), -(j % 256).astype(np.float64)])
    return r.astype(BF16)


def _const_ltb():
    ltb = np.zeros((2, 116), np.float64)
    ltb[0, 0:29] = 1.0
    ltb[1, 29:58] = 1.0
    ltb[0, 58:87] = 1.0
    ltb[1, 87:116] = 1.0
    return ltb.astype(BF16)


def _const_biasa():
    v = np.empty((128, 1), np.float32)
    v[0, 0] = 1.0
    v[127, 0] = 1.0
    for k in range(1, 127):
        v[k, 0] = 0.5 - k
    return v


def _const_biasb():
    v = np.empty((116, 1), np.float32)
    for k in range(29):
        v[k, 0] = -(k + 0.5)
        v[29 + k, 0] = -(k + 0.5)
    v[58:116] = v[0:58]
    return v


def kernel(mask, x_t, x_sc, W, b, gamma, beta):
    global LAST_PROFILE
    from concourse.bass_utils import run_bass_kernel_spmd

    mask = np.asarray(mask)
    cores, row_lists, R = _host_data(mask, x_t, x_sc, W, b)
    nc = _build_program(R, 128 - R)

    trace = bool(int(os.environ.get("KERNEL_TRACE", "0")))
    res = run_bass_kernel_spmd(nc, cores, list(range(N_CORES)), trace=trace)
    LAST_PROFILE = res

    out = np.empty((N, N, C_OUT), np.float32)
    for c in range(N_CORES):
        out[row_lists[c]] = res.results[c]["out"]

    gamma = np.asarray(gamma, np.float32)
    beta = np.asarray(beta, np.float32)
    if not (np.all(gamma == 1.0) and np.all(beta == 0.0)):
        pm = (mask.astype(np.float32)[:, None] * mask.astype(np.float32)[None, :])
        out = out * gamma[None, None, :] + pm[:, :, None] * beta[None, None, :]
    return out
